# revision 40
# baseline (speedup 1.0000x reference)
"""Trainium2 Bass kernel for a 2-layer GCN (nn_GCNModel).

Math (per GCNConv layer, PyG semantics):
    deg[d]  = sum_{e: dst=d} ew_e + 1                      (weighted in-degree + self loop)
    dinv    = deg^-1/2
    out[d]  = dinv[d] * ( sum_e  (ew_e * dinv[src_e]) * z[src_e] ) @ W + b
    where the edge list includes self loops (ew=1) and z is the layer input.

Key identity used: A_norm @ (z W) == (A_norm @ z) W  -- we aggregate the RAW
node features first, so layer 1 gathers straight from the input x, and only a
[dst x 128] @ [128 x C] matmul per output block applies W afterwards.

Distribution: dst-node sharding across 8 cores (12500 dsts each). Each core
gathers source rows from its own full replica of the feature table (x for
layer 1, h1 for layer 2 after an AllGather of the per-core h1 shards).

Per-core pipeline (per layer):
  - real edges sorted by (dst-block, src-chunk, dst); packed into 128-edge
    tiles; dma_gather pulls z[src] rows into SBUF, one edge per partition.
    The gathers round-robin over 4 SWDGE queues (queue_num=chunk): each queue
    is served by its own Q7 core pair, so up to 4 descriptor generations run
    concurrently (descgen on GPSIMD is the overall bottleneck at ~8ns/idx).
  - self loops never enter the gather stream: their rows are contiguous per
    block (own shard in dst order -> "selfx" input for layer 1, the local
    h1 shard for layer 2), loaded via zero-Q7-cost HWDGE dma_start and
    accumulated with a host-precomputed diagonal S ("s2diag") matmul.
  - DVE builds one-hot window matrices S[e, w] = (dstrel[e]==w) * weight[e]
  - PE accumulates U^T[ch, col] += M_tile^T @ S_tile into a PSUM bank
    (512 columns = one block of 500 dst nodes)
  - PE applies W (and the bias via a rank-1 matmul with sqrt(deg)), ACT does
    relu + the dinv[dst] scale fused, output rows stream to HBM.
  - the AllGather of h1 shard q fires right after its last contributing
    block (blocks 6/12/18/24), overlapping 3 of the 4 collectives with
    gather work; the last one stalls the Pool stream ~70us (Tile encodes
    collective deps as "all collectives emitted so far").

SPMD constraint: one instruction stream for all 8 cores, so the tile schedule
(tiles per group, PSUM column offset per tile) is computed jointly over all 8
cores ("consensus conveyor"), with per-core padding where a core has fewer
edges in a window.
"""

import os

import numpy as np
import ml_dtypes

N_NODES = 100000
N_EDGES = 1600000
IN_C, HID_C, OUT_C = 128, 128, 64

NC = 8
SHARD = N_NODES // NC          # 12500 dst nodes per core
BLK = 500                      # dst columns per PSUM accumulation block
NBLK = SHARD // BLK            # 25
SUB = 125                      # dst rows per stage-2 sub-block (4 per block)
NSUB = BLK // SUB              # 4
CHUNK = 25000                  # gather-table rows per int16 index chunk
NCHUNK = N_NODES // CHUNK      # 4
QCH = SHARD // NCHUNK          # 3125: per-core rows contributed to each chunk table
# node n lives in chunk table (n % SHARD) // QCH at row (n//SHARD)*QCH + n%QCH
# so each core's h1 shard maps to one contiguous 3125-row stripe per table,
# which is exactly what a per-table AllGather produces.
W = 64                         # one-hot window width (PSUM cols per tile)

USE_F32 = os.environ.get("GCN_F32", "0") == "1"
DBG_NBLK = int(os.environ.get("GCN_DBG_NBLK", "0"))      # 0 = all blocks
DBG_LAYERS = int(os.environ.get("GCN_DBG_LAYERS", "2"))  # 1 = layer 1 only
DBG_NO_AG = os.environ.get("GCN_DBG_NO_AG", "0") == "1"

LAST_RESULTS = None            # BassKernelResults of the most recent run
_CACHE = {}


# --------------------------------------------------------------------------
# host-side graph preprocessing
# --------------------------------------------------------------------------

def _preprocess(x, edge_index, edge_attr):
    src = np.ascontiguousarray(edge_index[0]).astype(np.int64)
    dst = np.ascontiguousarray(edge_index[1]).astype(np.int64)
    ew = np.ascontiguousarray(edge_attr).astype(np.float64)

    loop = np.arange(N_NODES, dtype=np.int64)
    deg = np.bincount(np.concatenate([dst, loop]),
                      weights=np.concatenate([ew, np.ones(N_NODES)]),
                      minlength=N_NODES)
    dinv = 1.0 / np.sqrt(deg)

    # Self loops are handled outside the gather stream (contiguous HWDGE
    # loads + a diagonal S matmul); only the real edges go through dma_gather.
    src_f, dst_f, ew_f = src, dst, ew
    wgt = (ew_f * dinv[src_f]).astype(np.float32)   # dinv[dst] applied post-agg

    core = dst_f // SHARD
    blk = (dst_f % SHARD) // BLK
    col = (dst_f % SHARD) % BLK
    ck = (src_f % SHARD) // QCH
    src_row = (src_f // SHARD) * QCH + (src_f % QCH)
    key = (core * NBLK + blk) * NCHUNK + ck
    order = np.lexsort((col, key))

    s_s = src_row[order].astype(np.int32)           # chunk-table row index
    col_s = col[order].astype(np.int32)
    w_s = wgt[order]
    key_s = key[order]

    # group starts: key values 0 .. NC*NBLK*NCHUNK-1
    ngroups = NC * NBLK * NCHUNK
    gstart = np.searchsorted(key_s, np.arange(ngroups + 1))

    # ---- consensus conveyor schedule over the 8 cores -------------------
    # sched[b][c] = list of window starts S_t (shared across cores)
    # tslice[j][b][c] = list of (lo, hi) edge ranges per tile for core j
    sched = [[None] * NCHUNK for _ in range(NBLK)]
    tslice = [[[None] * NCHUNK for _ in range(NBLK)] for _ in range(NC)]
    for b in range(NBLK):
        for c in range(NCHUNK):
            segs = []
            for j in range(NC):
                g = (j * NBLK + b) * NCHUNK + c
                segs.append((gstart[g], gstart[g + 1]))
            pos = [lo for lo, hi in segs]
            ends = [hi for lo, hi in segs]
            starts_list = []
            slices = [[] for _ in range(NC)]
            while True:
                cand = [col_s[pos[j]] for j in range(NC) if pos[j] < ends[j]]
                if not cand:
                    break
                st = int(min(cand))
                wt = min(W, BLK - st)
                endcol = st + wt
                starts_list.append(st)
                for j in range(NC):
                    if pos[j] < ends[j]:
                        hi = int(np.searchsorted(col_s[pos[j]:ends[j]], endcol)) + pos[j]
                        take = min(128, hi - pos[j])
                    else:
                        take = 0
                    slices[j].append((pos[j], pos[j] + take))
                    pos[j] += take
            sched[b][c] = starts_list
            for j in range(NC):
                tslice[j][b][c] = slices[j]

    nt_tot = sum(len(sched[b][c]) for b in range(NBLK) for c in range(NCHUNK))

    # ---- fill per-core packed arrays ------------------------------------
    idx_all = np.zeros((NC, nt_tot * 128), np.int16)
    dr_all = np.zeros((NC, nt_tot, 128), np.float32)
    ew_all = np.zeros((NC, nt_tot, 128), np.float32)
    t_glob = 0
    for b in range(NBLK):
        for c in range(NCHUNK):
            starts_list = sched[b][c]
            nt = len(starts_list)
            for ti, st in enumerate(starts_list):
                tg = t_glob + ti
                for j in range(NC):
                    lo, hi = tslice[j][b][c][ti]
                    n = hi - lo
                    if n == 0:
                        continue
                    base = tg * 128
                    idx_all[j, base : base + n] = s_s[lo:hi]
                    dr_all[j, tg, :n] = col_s[lo:hi] - st
                    ew_all[j, tg, :n] = w_s[lo:hi]
            # NOTE: pad slots must keep idx 0 (gather row 0, killed by S=0).
            # Trailing -1 trimming is UNSAFE here: the Q7 trims per-core
            # (data-driven) while the decode-side ring reservation uses the
            # static count, leaving stale ring descriptors the DMA executes.
            t_glob += nt

    # idx layout: index i at [i % 16, i // 16], replicated to 128 partitions
    idx16 = idx_all.reshape(NC, nt_tot * 8, 16).transpose(0, 2, 1)  # [NC,16,S]
    idx128 = np.tile(idx16, (1, 8, 1))                              # [NC,128,S]
    # dstrel/ew layout: edge slot p of tile t at [p, t]
    dr128 = dr_all.transpose(0, 2, 1)                               # [NC,128,NT]
    ew128 = ew_all.transpose(0, 2, 1)

    # stage-2 per-core tables
    dinv_f = dinv.astype(np.float32)
    sqdeg_f = np.sqrt(deg).astype(np.float32)
    dinv_cols = np.zeros((NC, 128, NBLK * NSUB), np.float32)
    sqdeg_rows = np.zeros((NC, 1, SHARD), np.float32)
    # diagonal S for self loops: s2[j, p, sb*SUB + q] = (p==q) * dinv[dst]
    s2_diag = np.zeros((NC, 128, NBLK * NSUB * SUB), np.float32)
    for j in range(NC):
        d = dinv_f[j * SHARD : (j + 1) * SHARD]
        dinv_cols[j, :SUB, :] = d.reshape(NBLK * NSUB, SUB).T
        sqdeg_rows[j, 0, :] = sqdeg_f[j * SHARD : (j + 1) * SHARD]
        for sb in range(NBLK * NSUB):
            seg = d[sb * SUB : (sb + 1) * SUB]
            s2_diag[j, np.arange(SUB), sb * SUB + np.arange(SUB)] = seg

    nt_a = sum(len(sched[b][c]) for b in range(2) for c in range(NCHUNK))
    return dict(
        sched=sched, nt_tot=nt_tot, nt_a=nt_a,
        idx128=idx128, dr128=dr128, ew128=ew128,
        dinv_cols=dinv_cols, sqdeg_rows=sqdeg_rows, s2_diag=s2_diag,
    )


# --------------------------------------------------------------------------
# device program
# --------------------------------------------------------------------------

def _build(sched, nt_tot):
    import concourse.bacc as bacc
    import concourse.tile as tile
    from concourse import mybir

    DT = mybir.dt.float32 if USE_F32 else mybir.dt.bfloat16

    nc = bacc.Bacc("TRN2", target_bir_lowering=False, debug=False,
                   num_devices=NC, num_swdge_queues=4,
                   dynamic_dma_scratch_size=32768)

    x_ins = [nc.dram_tensor(f"x{c}", [CHUNK, IN_C], DT, kind="ExternalInput")
             for c in range(NCHUNK)]
    w1_in = nc.dram_tensor("W1", [IN_C, HID_C], mybir.dt.float32, kind="ExternalInput")
    b1_in = nc.dram_tensor("b1", [1, HID_C], mybir.dt.float32, kind="ExternalInput")
    w2_in = nc.dram_tensor("W2", [HID_C, OUT_C], mybir.dt.float32, kind="ExternalInput")
    b2_in = nc.dram_tensor("b2", [1, OUT_C], mybir.dt.float32, kind="ExternalInput")
    # idx stream split: first 2 blocks' calls in a small tile loaded first so
    # the first gather doesn't wait for the full ~27KB/partition idx load.
    nt_a = sum(len(sched[b][c]) for b in range(2) for c in range(NCHUNK))
    idx_a_in = nc.dram_tensor("idx16a", [128, nt_a * 8], mybir.dt.int16, kind="ExternalInput")
    idx_b_in = nc.dram_tensor("idx16b", [128, (nt_tot - nt_a) * 8], mybir.dt.int16, kind="ExternalInput")
    dr_in = nc.dram_tensor("dstrel", [128, nt_tot], DT, kind="ExternalInput")
    ew_in = nc.dram_tensor("eww", [128, nt_tot], DT, kind="ExternalInput")
    iota_in = nc.dram_tensor("iota", [128, W], DT, kind="ExternalInput")
    dinv_in = nc.dram_tensor("dinvc", [128, NBLK * NSUB], mybir.dt.float32, kind="ExternalInput")
    sq_in = nc.dram_tensor("sqdeg", [1, SHARD], mybir.dt.float32, kind="ExternalInput")
    selfx_in = nc.dram_tensor("selfx", [SHARD, IN_C], DT, kind="ExternalInput")
    s2_in = nc.dram_tensor("s2diag", [128, NBLK * NSUB * SUB], DT, kind="ExternalInput")
    out_t = nc.dram_tensor("out", [SHARD, OUT_C], mybir.dt.float32, kind="ExternalOutput")

    ntmax = max(max(len(sched[b][c]) for c in range(NCHUNK)) for b in range(NBLK))

    with tile.TileContext(nc) as tc:
        with (
            tc.tile_pool(name="const", bufs=1) as cp,
            tc.tile_pool(name="mpool", bufs=12) as mp,
            tc.tile_pool(name="spool", bufs=10) as sp,
            tc.tile_pool(name="mself", bufs=8) as msp,
            tc.tile_pool(name="upool", bufs=2) as up,
            tc.tile_pool(name="hpool", bufs=4) as hp,
            tc.tile_pool(name="psU", bufs=5, space="PSUM") as ppu,
            tc.tile_pool(name="ps2", bufs=2, space="PSUM") as pp2,
            tc.tile_pool(name="dram", bufs=1, space="DRAM") as dp,
        ):
            idx_a_t = cp.tile([128, nt_a * 8], mybir.dt.int16)
            idx_b_t = cp.tile([128, (nt_tot - nt_a) * 8], mybir.dt.int16)
            dr_t = cp.tile([128, nt_tot], DT)
            ew_t = cp.tile([128, nt_tot], DT)
            iota_t = cp.tile([128, W], DT)
            dinv_t = cp.tile([128, NBLK * NSUB], mybir.dt.float32)
            sq_t = cp.tile([1, SHARD], mybir.dt.float32)
            s2_t = cp.tile([128, NBLK * NSUB * SUB], DT)
            w1_t = cp.tile([IN_C, HID_C], mybir.dt.float32)
            b1_t = cp.tile([1, HID_C], mybir.dt.float32)
            w2_t = cp.tile([HID_C, OUT_C], mybir.dt.float32)
            b2_t = cp.tile([1, OUT_C], mybir.dt.float32)
            zl_t = cp.tile([128, 128], DT)
            zr_t = cp.tile([128, 512], DT)

            for t, src in [(idx_a_t, idx_a_in), (dr_t, dr_in), (ew_t, ew_in),
                           (iota_t, iota_in), (dinv_t, dinv_in), (sq_t, sq_in),
                           (s2_t, s2_in),
                           (w1_t, w1_in), (b1_t, b1_in), (w2_t, w2_in), (b2_t, b2_in),
                           (idx_b_t, idx_b_in)]:
                nc.sync.dma_start(t[:], src[:])
            nc.vector.memset(zl_t[:], 0.0)
            nc.vector.memset(zr_t[:], 0.0)

            h1_shards = [dp.tile([QCH, HID_C], DT, name=f"h1s{c}", tag=f"h1s{c}") for c in range(NCHUNK)]
            h1_tables = [dp.tile([CHUNK, HID_C], DT, name=f"h1t{c}", tag=f"h1t{c}", addr_space="Shared") for c in range(NCHUNK)]

            nblk_run = DBG_NBLK if DBG_NBLK else NBLK
            for layer in range(DBG_LAYERS):
                tables = x_ins if layer == 0 else h1_tables
                wmat, brow = (w1_t, b1_t) if layer == 0 else (w2_t, b2_t)
                cout = HID_C if layer == 0 else OUT_C

                # AllGather shard q as soon as its last contributing block is
                # done (shard q holds sub-blocks [25q, 25q+25), sb = 4b+i).
                ag_after = {(25 * q + 24) // 4: q for q in range(NCHUNK)}

                t_glob = 0
                for b in range(nblk_run):
                    psU = ppu.tile([128, 512], mybir.dt.float32)
                    nc.tensor.matmul(psU[:], zl_t[:], zr_t[:], start=True, stop=False)
                    # last (c, tile) with any tiles, to set stop=True
                    lasts = [(c, len(sched[b][c]) - 1) for c in range(NCHUNK)
                             if len(sched[b][c]) > 0]
                    last_c, last_t = lasts[-1]
                    # self loops: contiguous rows via HWDGE + diagonal S
                    for i in range(NSUB):
                        sb = b * NSUB + i
                        ms = msp.tile([SUB, 128], DT, tag="ms")
                        # loads go on the Activation HWDGE queue to keep the
                        # Sync queue free for the h1-shard writes that gate
                        # the AllGather triggers
                        if layer == 0:
                            nc.scalar.dma_start(
                                ms[:], selfx_in[sb * SUB : (sb + 1) * SUB, :])
                        else:
                            q, lr = divmod(sb * SUB, QCH)
                            nc.scalar.dma_start(
                                ms[:], h1_shards[q][lr : lr + SUB, :])
                        nc.tensor.matmul(
                            psU[:, i * SUB : (i + 1) * SUB],
                            ms[:],
                            s2_t[0:SUB, sb * SUB : (sb + 1) * SUB],
                            start=False, stop=False,
                        )
                    for c in range(NCHUNK):
                        starts_list = sched[b][c]
                        nt = len(starts_list)
                        if nt == 0:
                            continue
                        src_ap = tables[c][:]
                        mb = mp.tile([128, ntmax, IN_C], DT, tag="mb")
                        if t_glob + nt <= nt_a:
                            idx_ap = idx_a_t[:, t_glob * 8 : (t_glob + nt) * 8]
                        else:
                            idx_ap = idx_b_t[:, (t_glob - nt_a) * 8 : (t_glob + nt - nt_a) * 8]
                        nc.gpsimd.dma_gather(
                            out_ap=mb[:, 0:nt, :],
                            in_ap=src_ap,
                            idxs_ap=idx_ap,
                            num_idxs=nt * 128,
                            num_idxs_reg=nt * 128,
                            elem_size=IN_C,
                            single_packet=False,
                            queue_num=c,
                        )
                        st_t = sp.tile([128, ntmax, W], DT, tag="st")
                        nc.vector.tensor_tensor(
                            out=st_t[:, 0:nt, :],
                            in0=dr_t[:, t_glob : t_glob + nt].unsqueeze(2).to_broadcast([128, nt, W]),
                            in1=iota_t[:].unsqueeze(1).to_broadcast([128, nt, W]),
                            op=mybir.AluOpType.is_equal,
                        )
                        nc.vector.tensor_mul(
                            out=st_t[:, 0:nt, :],
                            in0=st_t[:, 0:nt, :],
                            in1=ew_t[:, t_glob : t_glob + nt].unsqueeze(2).to_broadcast([128, nt, W]),
                        )
                        for ti, stc in enumerate(starts_list):
                            wt = min(W, BLK - stc)
                            nc.tensor.matmul(
                                psU[:, stc : stc + wt],
                                mb[:, ti, :],
                                st_t[:, ti, 0:wt],
                                start=False,
                                stop=(c == last_c and ti == last_t),
                            )
                        t_glob += nt

                    uT = up.tile([128, BLK], mybir.dt.float32)
                    nc.scalar.copy(uT[:], psU[:, 0:BLK])
                    for i in range(NSUB):
                        ps2 = pp2.tile([SUB, cout], mybir.dt.float32)
                        nc.tensor.matmul(ps2[:], uT[:, i * SUB : (i + 1) * SUB],
                                         wmat[:, 0:cout], start=True, stop=False)
                        nc.tensor.matmul(
                            ps2[:],
                            sq_t[0:1, b * BLK + i * SUB : b * BLK + (i + 1) * SUB],
                            brow[:, 0:cout], start=False, stop=True,
                        )
                        sb_idx = b * NSUB + i
                        if layer == 0:
                            ht = hp.tile([SUB, HID_C], DT, tag="ht")
                            nc.scalar.activation(
                                ht[:], ps2[:], mybir.ActivationFunctionType.Relu,
                                scale=dinv_t[0:SUB, sb_idx : sb_idx + 1],
                            )
                            q, lr = divmod(sb_idx * SUB, QCH)
                            nc.sync.dma_start(
                                h1_shards[q][lr : lr + SUB, :], ht[:],
                            )
                        else:
                            ot = hp.tile([SUB, OUT_C], mybir.dt.float32, tag="ot")
                            nc.scalar.activation(
                                ot[:], ps2[:], mybir.ActivationFunctionType.Copy,
                                scale=dinv_t[0:SUB, sb_idx : sb_idx + 1],
                            )
                            nc.sync.dma_start(
                                out_t[b * BLK + i * SUB : b * BLK + (i + 1) * SUB, :],
                                ot[:],
                            )

                    if layer == 0 and not DBG_NO_AG and b in ag_after:
                        q = ag_after[b]
                        nc.gpsimd.collective_compute(
                            "AllGather",
                            mybir.AluOpType.bypass,
                            replica_groups=[list(range(NC))],
                            ins=[h1_shards[q].opt()],
                            outs=[h1_tables[q].opt()],
                        )

    nc.compile()
    return nc


# --------------------------------------------------------------------------
# entry point
# --------------------------------------------------------------------------

def kernel(x, edge_index, edge_attr, W1, b1, W2, b2):
    global LAST_RESULTS
    import sys
    for p in ("/opt/trn_rl_repo",):
        if p not in sys.path:
            sys.path.insert(0, p)
    from concourse.bass_utils import run_bass_kernel_spmd

    x = np.asarray(x, dtype=np.float32)
    edge_index = np.asarray(edge_index)
    edge_attr = np.asarray(edge_attr, dtype=np.float32)
    W1 = np.asarray(W1, dtype=np.float32)
    b1 = np.asarray(b1, dtype=np.float32)
    W2 = np.asarray(W2, dtype=np.float32)
    b2 = np.asarray(b2, dtype=np.float32)

    import hashlib
    h = hashlib.sha1(edge_index.tobytes() + edge_attr.tobytes()).hexdigest()[:16]
    if h in _CACHE:
        nc, prep = _CACHE[h]
    else:
        prep = _preprocess(x, edge_index, edge_attr)
        nc = _build(prep["sched"], prep["nt_tot"])
        _CACHE[h] = (nc, prep)

    np_dt = np.float32 if USE_F32 else ml_dtypes.bfloat16
    n_all = np.arange(N_NODES)
    c_arr = (n_all % SHARD) // QCH
    r_arr = (n_all // SHARD) * QCH + (n_all % QCH)
    x_t = x.astype(np_dt)
    x_chunks = []
    for c in range(NCHUNK):
        xc = np.zeros((CHUNK, IN_C), np_dt)
        m = c_arr == c
        xc[r_arr[m]] = x_t[m]
        x_chunks.append(xc)
    iota = np.tile(np.arange(W, dtype=np.float32), (128, 1)).astype(np_dt)

    in_maps = []
    for j in range(NC):
        im = {f"x{c}": x_chunks[c] for c in range(NCHUNK)}
        in_maps.append({
            **im,
            "W1": W1, "b1": b1.reshape(1, HID_C),
            "W2": W2, "b2": b2.reshape(1, OUT_C),
            "idx16a": np.ascontiguousarray(prep["idx128"][j][:, : prep["nt_a"] * 8]),
            "idx16b": np.ascontiguousarray(prep["idx128"][j][:, prep["nt_a"] * 8 :]),
            "dstrel": prep["dr128"][j].astype(np_dt),
            "eww": prep["ew128"][j].astype(np_dt),
            "iota": iota,
            "dinvc": prep["dinv_cols"][j],
            "sqdeg": prep["sqdeg_rows"][j],
            "selfx": np.ascontiguousarray(x_t[j * SHARD : (j + 1) * SHARD]),
            "s2diag": prep["s2_diag"][j].astype(np_dt),
        })

    trace = os.environ.get("GCN_TRACE", "0") == "1"
    res = run_bass_kernel_spmd(nc, in_maps, core_ids=list(range(NC)),
                               trace=trace)
    LAST_RESULTS = res
    out = np.concatenate([res.results[j]["out"] for j in range(NC)], axis=0)
    return out.astype(np.float32)



# revision 41
# speedup vs baseline: 1.1802x; 1.1802x over previous
"""Trainium2 Bass kernel for a 2-layer GCN (nn_GCNModel).

Math (per GCNConv layer, PyG semantics):
    deg[d]  = sum_{e: dst=d} ew_e + 1                      (weighted in-degree + self loop)
    dinv    = deg^-1/2
    out[d]  = dinv[d] * ( sum_e  (ew_e * dinv[src_e]) * z[src_e] ) @ W + b
    where the edge list includes self loops (ew=1) and z is the layer input.

Key identity used: A_norm @ (z W) == (A_norm @ z) W  -- we aggregate the RAW
node features first, so layer 1 gathers straight from the input x, and only a
[dst x 128] @ [128 x C] matmul per output block applies W afterwards.

Distribution: dst-node sharding across 8 cores (12500 dsts each). Each core
gathers source rows from its own full replica of the feature table (x for
layer 1, h1 for layer 2 after an AllGather of the per-core h1 shards).

Per-core pipeline (per layer):
  - real edges sorted by (dst-block, src-chunk, dst); packed into 128-edge
    tiles; dma_gather pulls z[src] rows into SBUF, one edge per partition.
    The gathers round-robin over 4 SWDGE queues (queue_num=chunk): each queue
    is served by its own Q7 core pair, so up to 4 descriptor generations run
    concurrently (descgen on GPSIMD is the overall bottleneck at ~8ns/idx).
  - self loops never enter the gather stream: their rows are contiguous per
    block (own shard in dst order -> "selfx" input for layer 1, the local
    h1 shard for layer 2), loaded via zero-Q7-cost HWDGE dma_start and
    accumulated with a host-precomputed diagonal S ("s2diag") matmul.
  - DVE builds one-hot window matrices S[e, w] = (dstrel[e]==w) * weight[e]
  - PE accumulates U^T[ch, col] += M_tile^T @ S_tile into a PSUM bank
    (512 columns = one block of 500 dst nodes)
  - PE applies W (and the bias via a rank-1 matmul with sqrt(deg)), ACT does
    relu + the dinv[dst] scale fused, output rows stream to HBM.
  - the AllGather of h1 shard q fires right after its last contributing
    block (blocks 6/12/18/24), overlapping 3 of the 4 collectives with
    gather work; the last one stalls the Pool stream ~70us (Tile encodes
    collective deps as "all collectives emitted so far").

SPMD constraint: one instruction stream for all 8 cores, so the tile schedule
(tiles per group, PSUM column offset per tile) is computed jointly over all 8
cores ("consensus conveyor"), with per-core padding where a core has fewer
edges in a window.
"""

import os

import numpy as np
import ml_dtypes

N_NODES = 100000
N_EDGES = 1600000
IN_C, HID_C, OUT_C = 128, 128, 64

NC = 8
SHARD = N_NODES // NC          # 12500 dst nodes per core
BLK = 500                      # dst columns per PSUM accumulation block
NBLK = SHARD // BLK            # 25
SUB = 125                      # dst rows per stage-2 sub-block (4 per block)
NSUB = BLK // SUB              # 4
CHUNK = 25000                  # gather-table rows per int16 index chunk
NCHUNK = N_NODES // CHUNK      # 4
QCH = SHARD // NCHUNK          # 3125: per-core rows contributed to each chunk table
# node n lives in chunk table (n % SHARD) // QCH at row (n//SHARD)*QCH + n%QCH
# so each core's h1 shard maps to one contiguous 3125-row stripe per table,
# which is exactly what a per-table AllGather produces.
W = 64                         # one-hot window width (PSUM cols per tile)

USE_F32 = os.environ.get("GCN_F32", "0") == "1"
DBG_NBLK = int(os.environ.get("GCN_DBG_NBLK", "0"))      # 0 = all blocks
DBG_LAYERS = int(os.environ.get("GCN_DBG_LAYERS", "2"))  # 1 = layer 1 only
DBG_NO_AG = os.environ.get("GCN_DBG_NO_AG", "0") == "1"

LAST_RESULTS = None            # BassKernelResults of the most recent run
_CACHE = {}


# --------------------------------------------------------------------------
# host-side graph preprocessing
# --------------------------------------------------------------------------

def _preprocess(x, edge_index, edge_attr):
    src = np.ascontiguousarray(edge_index[0]).astype(np.int64)
    dst = np.ascontiguousarray(edge_index[1]).astype(np.int64)
    ew = np.ascontiguousarray(edge_attr).astype(np.float64)

    loop = np.arange(N_NODES, dtype=np.int64)
    deg = np.bincount(np.concatenate([dst, loop]),
                      weights=np.concatenate([ew, np.ones(N_NODES)]),
                      minlength=N_NODES)
    dinv = 1.0 / np.sqrt(deg)

    # Self loops are handled outside the gather stream (contiguous HWDGE
    # loads + a diagonal S matmul); only the real edges go through dma_gather.
    src_f, dst_f, ew_f = src, dst, ew
    wgt = (ew_f * dinv[src_f]).astype(np.float32)   # dinv[dst] applied post-agg

    core = dst_f // SHARD
    blk = (dst_f % SHARD) // BLK
    col = (dst_f % SHARD) % BLK
    ck = (src_f % SHARD) // QCH
    src_row = (src_f // SHARD) * QCH + (src_f % QCH)
    key = (core * NBLK + blk) * NCHUNK + ck
    order = np.lexsort((col, key))

    s_s = src_row[order].astype(np.int32)           # chunk-table row index
    col_s = col[order].astype(np.int32)
    w_s = wgt[order]
    key_s = key[order]

    # group starts: key values 0 .. NC*NBLK*NCHUNK-1
    ngroups = NC * NBLK * NCHUNK
    gstart = np.searchsorted(key_s, np.arange(ngroups + 1))

    # ---- consensus conveyor schedule over the 8 cores -------------------
    # sched[b][c] = list of window starts S_t (shared across cores)
    # tslice[j][b][c] = list of (lo, hi) edge ranges per tile for core j
    sched = [[None] * NCHUNK for _ in range(NBLK)]
    tslice = [[[None] * NCHUNK for _ in range(NBLK)] for _ in range(NC)]
    for b in range(NBLK):
        for c in range(NCHUNK):
            segs = []
            for j in range(NC):
                g = (j * NBLK + b) * NCHUNK + c
                segs.append((gstart[g], gstart[g + 1]))
            pos = [lo for lo, hi in segs]
            ends = [hi for lo, hi in segs]
            starts_list = []
            slices = [[] for _ in range(NC)]
            while True:
                cand = [col_s[pos[j]] for j in range(NC) if pos[j] < ends[j]]
                if not cand:
                    break
                st = int(min(cand))
                wt = min(W, BLK - st)
                endcol = st + wt
                starts_list.append(st)
                for j in range(NC):
                    if pos[j] < ends[j]:
                        hi = int(np.searchsorted(col_s[pos[j]:ends[j]], endcol)) + pos[j]
                        take = min(128, hi - pos[j])
                    else:
                        take = 0
                    slices[j].append((pos[j], pos[j] + take))
                    pos[j] += take
            sched[b][c] = starts_list
            for j in range(NC):
                tslice[j][b][c] = slices[j]

    nt_tot = sum(len(sched[b][c]) for b in range(NBLK) for c in range(NCHUNK))

    # ---- fill per-core packed arrays ------------------------------------
    idx_all = np.zeros((NC, nt_tot * 128), np.int16)
    dr_all = np.zeros((NC, nt_tot, 128), np.float32)
    ew_all = np.zeros((NC, nt_tot, 128), np.float32)
    t_glob = 0
    for b in range(NBLK):
        for c in range(NCHUNK):
            starts_list = sched[b][c]
            nt = len(starts_list)
            for ti, st in enumerate(starts_list):
                tg = t_glob + ti
                for j in range(NC):
                    lo, hi = tslice[j][b][c][ti]
                    n = hi - lo
                    if n == 0:
                        continue
                    base = tg * 128
                    idx_all[j, base : base + n] = s_s[lo:hi]
                    dr_all[j, tg, :n] = col_s[lo:hi] - st
                    ew_all[j, tg, :n] = w_s[lo:hi]
            # NOTE: pad slots must keep idx 0 (gather row 0, killed by S=0).
            # Trailing -1 trimming is UNSAFE here: the Q7 trims per-core
            # (data-driven) while the decode-side ring reservation uses the
            # static count, leaving stale ring descriptors the DMA executes.
            t_glob += nt

    # idx layout: index i at [i % 16, i // 16], replicated to 128 partitions
    idx16 = idx_all.reshape(NC, nt_tot * 8, 16).transpose(0, 2, 1)  # [NC,16,S]
    idx128 = np.tile(idx16, (1, 8, 1))                              # [NC,128,S]
    # dstrel/ew layout: edge slot p of tile t at [p, t]
    dr128 = dr_all.transpose(0, 2, 1)                               # [NC,128,NT]
    ew128 = ew_all.transpose(0, 2, 1)

    # stage-2 per-core tables
    dinv_f = dinv.astype(np.float32)
    sqdeg_f = np.sqrt(deg).astype(np.float32)
    dinv_cols = np.zeros((NC, 128, NBLK * NSUB), np.float32)
    sqdeg_rows = np.zeros((NC, 1, SHARD), np.float32)
    # diagonal S for self loops: s2[j, p, sb*SUB + q] = (p==q) * dinv[dst]
    s2_diag = np.zeros((NC, 128, NBLK * NSUB * SUB), np.float32)
    for j in range(NC):
        d = dinv_f[j * SHARD : (j + 1) * SHARD]
        dinv_cols[j, :SUB, :] = d.reshape(NBLK * NSUB, SUB).T
        sqdeg_rows[j, 0, :] = sqdeg_f[j * SHARD : (j + 1) * SHARD]
        for sb in range(NBLK * NSUB):
            seg = d[sb * SUB : (sb + 1) * SUB]
            s2_diag[j, np.arange(SUB), sb * SUB + np.arange(SUB)] = seg

    nt_a = sum(len(sched[b][c]) for b in range(2) for c in range(NCHUNK))
    return dict(
        sched=sched, nt_tot=nt_tot, nt_a=nt_a,
        idx128=idx128, dr128=dr128, ew128=ew128,
        dinv_cols=dinv_cols, sqdeg_rows=sqdeg_rows, s2_diag=s2_diag,
    )


# --------------------------------------------------------------------------
# device program
# --------------------------------------------------------------------------

def _build(sched, nt_tot):
    import concourse.bacc as bacc
    import concourse.tile as tile
    from concourse import mybir

    DT = mybir.dt.float32 if USE_F32 else mybir.dt.bfloat16

    nc = bacc.Bacc("TRN2", target_bir_lowering=False, debug=False,
                   num_devices=NC, num_swdge_queues=4,
                   dynamic_dma_scratch_size=32768)

    x_ins = [nc.dram_tensor(f"x{c}", [CHUNK, IN_C], DT, kind="ExternalInput")
             for c in range(NCHUNK)]
    w1_in = nc.dram_tensor("W1", [IN_C, HID_C], mybir.dt.float32, kind="ExternalInput")
    b1_in = nc.dram_tensor("b1", [1, HID_C], mybir.dt.float32, kind="ExternalInput")
    w2_in = nc.dram_tensor("W2", [HID_C, OUT_C], mybir.dt.float32, kind="ExternalInput")
    b2_in = nc.dram_tensor("b2", [1, OUT_C], mybir.dt.float32, kind="ExternalInput")
    # idx stream split: first 2 blocks' calls in a small tile loaded first so
    # the first gather doesn't wait for the full ~27KB/partition idx load.
    nt_a = sum(len(sched[b][c]) for b in range(2) for c in range(NCHUNK))
    idx_a_in = nc.dram_tensor("idx16a", [128, nt_a * 8], mybir.dt.int16, kind="ExternalInput")
    idx_b_in = nc.dram_tensor("idx16b", [128, (nt_tot - nt_a) * 8], mybir.dt.int16, kind="ExternalInput")
    dr_in = nc.dram_tensor("dstrel", [128, nt_tot], DT, kind="ExternalInput")
    ew_in = nc.dram_tensor("eww", [128, nt_tot], DT, kind="ExternalInput")
    iota_in = nc.dram_tensor("iota", [128, W], DT, kind="ExternalInput")
    dinv_in = nc.dram_tensor("dinvc", [128, NBLK * NSUB], mybir.dt.float32, kind="ExternalInput")
    sq_in = nc.dram_tensor("sqdeg", [1, SHARD], mybir.dt.float32, kind="ExternalInput")
    selfx_in = nc.dram_tensor("selfx", [SHARD, IN_C], DT, kind="ExternalInput")
    s2_in = nc.dram_tensor("s2diag", [128, NBLK * NSUB * SUB], DT, kind="ExternalInput")
    out_t = nc.dram_tensor("out", [SHARD, OUT_C], mybir.dt.float32, kind="ExternalOutput")

    ntmax = max(max(len(sched[b][c]) for c in range(NCHUNK)) for b in range(NBLK))

    with tile.TileContext(nc) as tc:
        with (
            tc.tile_pool(name="const", bufs=1) as cp,
            tc.tile_pool(name="mpool", bufs=12) as mp,
            tc.tile_pool(name="spool", bufs=6) as sp,
            tc.tile_pool(name="mself", bufs=6) as msp,
            tc.tile_pool(name="upool", bufs=2) as up,
            tc.tile_pool(name="hpool", bufs=4) as hp,
            tc.tile_pool(name="psU", bufs=5, space="PSUM") as ppu,
            tc.tile_pool(name="ps2", bufs=2, space="PSUM") as pp2,
            tc.tile_pool(name="dram", bufs=1, space="DRAM") as dp,
        ):
            idx_a_t = cp.tile([128, nt_a * 8], mybir.dt.int16)
            idx_b_t = cp.tile([128, (nt_tot - nt_a) * 8], mybir.dt.int16)
            dr_t = cp.tile([128, nt_tot], DT)
            ew_t = cp.tile([128, nt_tot], DT)
            iota_t = cp.tile([128, W], DT)
            dinv_t = cp.tile([128, NBLK * NSUB], mybir.dt.float32)
            sq_t = cp.tile([1, SHARD], mybir.dt.float32)
            s2_t = cp.tile([128, NBLK * NSUB * SUB], DT)
            w1_t = cp.tile([IN_C, HID_C], mybir.dt.float32)
            b1_t = cp.tile([1, HID_C], mybir.dt.float32)
            w2_t = cp.tile([HID_C, OUT_C], mybir.dt.float32)
            b2_t = cp.tile([1, OUT_C], mybir.dt.float32)
            zl_t = cp.tile([128, 128], DT)
            zr_t = cp.tile([128, 512], DT)

            for t, src in [(idx_a_t, idx_a_in), (dr_t, dr_in), (ew_t, ew_in),
                           (iota_t, iota_in), (dinv_t, dinv_in), (sq_t, sq_in),
                           (s2_t, s2_in),
                           (w1_t, w1_in), (b1_t, b1_in), (w2_t, w2_in), (b2_t, b2_in),
                           (idx_b_t, idx_b_in)]:
                nc.sync.dma_start(t[:], src[:])
            nc.vector.memset(zl_t[:], 0.0)
            nc.vector.memset(zr_t[:], 0.0)

            h1_shards = [dp.tile([QCH, HID_C], DT, name=f"h1s{c}", tag=f"h1s{c}") for c in range(NCHUNK)]
            h1_tables = [dp.tile([CHUNK, HID_C], DT, name=f"h1t{c}", tag=f"h1t{c}", addr_space="Shared") for c in range(NCHUNK)]

            nblk_run = DBG_NBLK if DBG_NBLK else NBLK
            for layer in range(DBG_LAYERS):
                tables = x_ins if layer == 0 else h1_tables
                wmat, brow = (w1_t, b1_t) if layer == 0 else (w2_t, b2_t)
                cout = HID_C if layer == 0 else OUT_C

                # AllGather shard q as soon as its last contributing block is
                # done (shard q holds sub-blocks [25q, 25q+25), sb = 4b+i).
                ag_after = {(25 * q + 24) // 4: q for q in range(NCHUNK)}

                t_glob = 0
                for b in range(nblk_run):
                    psU = ppu.tile([128, 512], mybir.dt.float32)
                    nc.tensor.matmul(psU[:], zl_t[:], zr_t[:], start=True, stop=False)
                    # last (c, tile) with any tiles, to set stop=True
                    lasts = [(c, len(sched[b][c]) - 1) for c in range(NCHUNK)
                             if len(sched[b][c]) > 0]
                    last_c, last_t = lasts[-1]
                    # self loops: contiguous rows via HWDGE + diagonal S
                    for i in range(NSUB):
                        sb = b * NSUB + i
                        ms = msp.tile([SUB, 128], DT, tag="ms")
                        # loads go on the Activation HWDGE queue to keep the
                        # Sync queue free for the h1-shard writes that gate
                        # the AllGather triggers
                        if layer == 0:
                            nc.scalar.dma_start(
                                ms[:], selfx_in[sb * SUB : (sb + 1) * SUB, :])
                        else:
                            q, lr = divmod(sb * SUB, QCH)
                            nc.scalar.dma_start(
                                ms[:], h1_shards[q][lr : lr + SUB, :])
                        nc.tensor.matmul(
                            psU[:, i * SUB : (i + 1) * SUB],
                            ms[:],
                            s2_t[0:SUB, sb * SUB : (sb + 1) * SUB],
                            start=False, stop=False,
                        )
                    for c in range(NCHUNK):
                        starts_list = sched[b][c]
                        nt = len(starts_list)
                        if nt == 0:
                            continue
                        src_ap = tables[c][:]
                        mb = mp.tile([128, ntmax, IN_C], DT, tag="mb")
                        if t_glob + nt <= nt_a:
                            idx_ap = idx_a_t[:, t_glob * 8 : (t_glob + nt) * 8]
                        else:
                            idx_ap = idx_b_t[:, (t_glob - nt_a) * 8 : (t_glob + nt - nt_a) * 8]
                        nc.gpsimd.dma_gather(
                            out_ap=mb[:, 0:nt, :],
                            in_ap=src_ap,
                            idxs_ap=idx_ap,
                            num_idxs=nt * 128,
                            num_idxs_reg=nt * 128,
                            elem_size=IN_C,
                            single_packet=False,
                            queue_num=c,
                        )
                        st_t = sp.tile([128, ntmax, W], DT, tag="st")
                        nc.vector.tensor_tensor(
                            out=st_t[:, 0:nt, :],
                            in0=dr_t[:, t_glob : t_glob + nt].unsqueeze(2).to_broadcast([128, nt, W]),
                            in1=iota_t[:].unsqueeze(1).to_broadcast([128, nt, W]),
                            op=mybir.AluOpType.is_equal,
                        )
                        nc.vector.tensor_mul(
                            out=st_t[:, 0:nt, :],
                            in0=st_t[:, 0:nt, :],
                            in1=ew_t[:, t_glob : t_glob + nt].unsqueeze(2).to_broadcast([128, nt, W]),
                        )
                        for ti, stc in enumerate(starts_list):
                            wt = min(W, BLK - stc)
                            nc.tensor.matmul(
                                psU[:, stc : stc + wt],
                                mb[:, ti, :],
                                st_t[:, ti, 0:wt],
                                start=False,
                                stop=(c == last_c and ti == last_t),
                            )
                        t_glob += nt

                    uT = up.tile([128, BLK], mybir.dt.float32)
                    nc.scalar.copy(uT[:], psU[:, 0:BLK])
                    for i in range(NSUB):
                        ps2 = pp2.tile([SUB, cout], mybir.dt.float32)
                        nc.tensor.matmul(ps2[:], uT[:, i * SUB : (i + 1) * SUB],
                                         wmat[:, 0:cout], start=True, stop=False)
                        nc.tensor.matmul(
                            ps2[:],
                            sq_t[0:1, b * BLK + i * SUB : b * BLK + (i + 1) * SUB],
                            brow[:, 0:cout], start=False, stop=True,
                        )
                        sb_idx = b * NSUB + i
                        if layer == 0:
                            ht = hp.tile([SUB, HID_C], DT, tag="ht")
                            nc.scalar.activation(
                                ht[:], ps2[:], mybir.ActivationFunctionType.Relu,
                                scale=dinv_t[0:SUB, sb_idx : sb_idx + 1],
                            )
                            q, lr = divmod(sb_idx * SUB, QCH)
                            nc.sync.dma_start(
                                h1_shards[q][lr : lr + SUB, :], ht[:],
                            )
                        else:
                            ot = hp.tile([SUB, OUT_C], mybir.dt.float32, tag="ot")
                            nc.scalar.activation(
                                ot[:], ps2[:], mybir.ActivationFunctionType.Copy,
                                scale=dinv_t[0:SUB, sb_idx : sb_idx + 1],
                            )
                            nc.sync.dma_start(
                                out_t[b * BLK + i * SUB : b * BLK + (i + 1) * SUB, :],
                                ot[:],
                            )

                    if layer == 0 and not DBG_NO_AG and b in ag_after:
                        q = ag_after[b]
                        nc.gpsimd.collective_compute(
                            "AllGather",
                            mybir.AluOpType.bypass,
                            replica_groups=[list(range(NC))],
                            ins=[h1_shards[q].opt()],
                            outs=[h1_tables[q].opt()],
                        )

    nc.compile()
    return nc


# --------------------------------------------------------------------------
# entry point
# --------------------------------------------------------------------------

def kernel(x, edge_index, edge_attr, W1, b1, W2, b2):
    global LAST_RESULTS
    import sys
    for p in ("/opt/trn_rl_repo",):
        if p not in sys.path:
            sys.path.insert(0, p)
    from concourse.bass_utils import run_bass_kernel_spmd

    x = np.asarray(x, dtype=np.float32)
    edge_index = np.asarray(edge_index)
    edge_attr = np.asarray(edge_attr, dtype=np.float32)
    W1 = np.asarray(W1, dtype=np.float32)
    b1 = np.asarray(b1, dtype=np.float32)
    W2 = np.asarray(W2, dtype=np.float32)
    b2 = np.asarray(b2, dtype=np.float32)

    import hashlib
    h = hashlib.sha1(edge_index.tobytes() + edge_attr.tobytes()).hexdigest()[:16]
    if h in _CACHE:
        nc, prep = _CACHE[h]
    else:
        prep = _preprocess(x, edge_index, edge_attr)
        nc = _build(prep["sched"], prep["nt_tot"])
        _CACHE[h] = (nc, prep)

    np_dt = np.float32 if USE_F32 else ml_dtypes.bfloat16
    n_all = np.arange(N_NODES)
    c_arr = (n_all % SHARD) // QCH
    r_arr = (n_all // SHARD) * QCH + (n_all % QCH)
    x_t = x.astype(np_dt)
    x_chunks = []
    for c in range(NCHUNK):
        xc = np.zeros((CHUNK, IN_C), np_dt)
        m = c_arr == c
        xc[r_arr[m]] = x_t[m]
        x_chunks.append(xc)
    iota = np.tile(np.arange(W, dtype=np.float32), (128, 1)).astype(np_dt)

    in_maps = []
    for j in range(NC):
        im = {f"x{c}": x_chunks[c] for c in range(NCHUNK)}
        in_maps.append({
            **im,
            "W1": W1, "b1": b1.reshape(1, HID_C),
            "W2": W2, "b2": b2.reshape(1, OUT_C),
            "idx16a": np.ascontiguousarray(prep["idx128"][j][:, : prep["nt_a"] * 8]),
            "idx16b": np.ascontiguousarray(prep["idx128"][j][:, prep["nt_a"] * 8 :]),
            "dstrel": prep["dr128"][j].astype(np_dt),
            "eww": prep["ew128"][j].astype(np_dt),
            "iota": iota,
            "dinvc": prep["dinv_cols"][j],
            "sqdeg": prep["sqdeg_rows"][j],
            "selfx": np.ascontiguousarray(x_t[j * SHARD : (j + 1) * SHARD]),
            "s2diag": prep["s2_diag"][j].astype(np_dt),
        })

    trace = os.environ.get("GCN_TRACE", "0") == "1"
    res = run_bass_kernel_spmd(nc, in_maps, core_ids=list(range(NC)),
                               trace=trace)
    LAST_RESULTS = res
    out = np.concatenate([res.results[j]["out"] for j in range(NC)], axis=0)
    return out.astype(np.float32)

